# revision 1
# baseline (speedup 1.0000x reference)
"""Multi-head attention (B=2, S=2048, H=1024, 16 heads) on 8 TRN2 NeuronCores.

Sharding (tensor-parallel heads x data-parallel batch, per the hint):
  core c -> batch b = c // 4, head group g = c % 4 (4 heads each).
Each core computes, for its batch and its 4 heads:
  Q^T, K^T (transposed layout, qcol on partitions, fp16, duplicated across
  both partition halves), V^T (f32r) transposed to natural V (bf16) on the
  PE, scores^T = K^T.T @ Q^T per head with two tok_k chunks row-tiled
  concurrently in the two array halves, probs = exp(scores) (no max
  subtraction -- scores ~ N(0,1), bounded), ctx^T via an M=65 ones-augmented
  V so the softmax denominator lands in row 64 of the same accumulation,
  division by the denominator (reciprocal + gpsimd partition broadcast),
  and the partial out-projection ctx^T.T @ Wo_rows.  The 4 partial outputs
  per batch are summed on the host during unsharding (Megatron-style TP
  partial sums).

Projections run as float32r (full PE rate at N>=512); the scores operands
are fp16 and the probs/V path is bf16 (all well inside the tolerance).
Biases: bq/bk are applied on-device (per-partition adds; they are zeros in
this problem), bv/bo are folded into a host-side additive constant
(bv @ Wo + bo), which is exact.
"""

import ml_dtypes
import numpy as np

import concourse.bacc as bacc
import concourse.mybir as mybir
import concourse.tile as tile
from concourse.bass_utils import run_bass_kernel_spmd

NCORES = 8
B, S, HID = 2, 2048, 1024
NH, HD = 16, 64
HPC = 4            # heads per core
QC = HPC * HD      # 256 local projection cols per core
HC = HID // 128    # 8 hidden chunks
TC = S // 128      # 16 token chunks
TB = S // 512      # 4 token blocks

F32 = mybir.dt.float32
F32R = mybir.dt.float32r
BF16 = mybir.dt.bfloat16
FP16 = mybir.dt.float16
EXP = mybir.ActivationFunctionType.Exp
MULT = mybir.AluOpType.mult


def build_nc():
    nc = bacc.Bacc("TRN2", target_bir_lowering=False, debug=False,
                   num_devices=NCORES)
    xT = nc.declare_dram_parameter("xT", [HID, S], FP16, isOutput=False)
    wq = nc.declare_dram_parameter("wq", [HID, QC], FP16, isOutput=False)
    wk = nc.declare_dram_parameter("wk", [HID, QC], FP16, isOutput=False)
    wv = nc.declare_dram_parameter("wv", [HID, QC], FP16, isOutput=False)
    wo = nc.declare_dram_parameter("wo", [QC, HID], BF16, isOutput=False)
    bq = nc.declare_dram_parameter("bq", [QC], F32, isOutput=False)
    bk = nc.declare_dram_parameter("bk", [QC], F32, isOutput=False)
    ident = nc.declare_dram_parameter("ident", [128, 128], F32, isOutput=False)
    out = nc.declare_dram_parameter("out", [S, HID], BF16, isOutput=True)

    with tile.TileContext(nc) as tc:
        with (
            tc.tile_pool(name="const", bufs=1) as constp,
            tc.tile_pool(name="qkv", bufs=1) as qkvp,
        ):
            wo_sb = constp.tile([128, 2 * HID], BF16)
            bq_sb = constp.tile([128, 2], F32)
            bk_sb = constp.tile([128, 2], F32)
            idf_sb = constp.tile([128, 128], F32)
            idb_sb = constp.tile([128, 128], BF16)
            one_f32 = constp.tile([1, 128], F32)
            one_sb = constp.tile([1, 128], F32R)
            nc.vector.memset(one_f32[:], 1.0)
            nc.vector.tensor_copy(one_sb[:], one_f32[:])
            # Q^T/K^T per head, duplicated across both partition halves:
            # head h occupies free range [h*S, (h+1)*S) with the same [64, S]
            # data in partitions 0-63 and 64-127, so the scores matmuls can
            # run two tok_k chunks concurrently as row-tiles.
            qt2 = qkvp.tile([128, HPC * S], FP16)
            kt2 = qkvp.tile([128, HPC * S], FP16)
            # Natural V (bf16) with a ones column at col 64 of each 128-wide
            # per-head strip: the ctx matmul's M=65 stationary computes ctx
            # rows 0..63 plus the softmax denominator in row 64.
            v_sb = qkvp.tile([128, TC * HPC * 128], BF16)
            vt_sb = qkvp.tile([128, 2 * S], BF16)
            ctxf_sb = qkvp.tile([128, 2 * S], BF16)

            for t in range(TC):
                for h in range(HPC):
                    off = (t * HPC + h) * 128 + HD
                    nc.vector.memset(v_sb[:, off:off + 1], 1.0)


            # ---- phase 1: projections -------------------------------------
            with (
                tc.tile_pool(name="xw", bufs=1) as xwp,
                tc.tile_pool(name="ps1", bufs=2, space="PSUM") as ps1,
            ):
                xT_sb = xwp.tile([128, HC * S], FP16)
                wq_sb = xwp.tile([128, HC * QC], FP16)
                wk_sb = xwp.tile([128, HC * QC], FP16)
                wv_sb = xwp.tile([128, HC * QC], FP16)
                # wv + the first two xT chunks get the DMA engines to
                # themselves; later inputs are paced behind early V^T
                # matmuls (add_dep_helper) so the first compute isn't stuck
                # behind the whole 6 MB input load.
                xt_dmas = {}
                for hc in range(HC):
                    r = slice(hc * 128, (hc + 1) * 128)
                    nc.scalar.dma_start(wv_sb[:, hc * QC:(hc + 1) * QC],
                                        wv[r, :])
                    eng = nc.sync if hc % 2 == 0 else nc.scalar
                    if hc == 0:
                        # j-quartered so the first V^T matmul (which reads
                        # only tokens 0..511 of chunk 0) starts asap
                        for j in range(TB):
                            xt_dmas[hc] = eng.dma_start(
                                xT_sb[:, hc * S + j * 512:hc * S + (j + 1) * 512],
                                xT[r, j * 512:(j + 1) * 512])
                    else:
                        xt_dmas[hc] = eng.dma_start(
                            xT_sb[:, hc * S:(hc + 1) * S], xT[r, :])
                nc.scalar.dma_start(idf_sb[:, :], ident[:, :])
                nc.vector.tensor_copy(idb_sb[:, :], idf_sb[:, :])
                for ci in range(2):
                    nc.sync.dma_start(bq_sb[:, ci:ci + 1],
                                      bq[ci * 128:(ci + 1) * 128])
                    nc.sync.dma_start(bk_sb[:, ci:ci + 1],
                                      bk[ci * 128:(ci + 1) * 128])
                qk_dmas = []
                for hc in range(HC):
                    r = slice(hc * 128, (hc + 1) * 128)
                    qk_dmas.append(nc.sync.dma_start(
                        wq_sb[:, hc * QC:(hc + 1) * QC], wq[r, :]))
                    qk_dmas.append(nc.sync.dma_start(
                        wk_sb[:, hc * QC:(hc + 1) * QC], wk[r, :]))

                # V^T first (kept in SBUF; transposed on the PE below)
                vt_mms = {}
                for ci in range(2):
                    ps = ps1.tile([128, S], F32, tag="ps1")
                    for hc in range(HC):
                        for j in range(TB):
                            mm = nc.tensor.matmul(
                                ps[:, j * 512:(j + 1) * 512],
                                wv_sb[:, hc * QC + ci * 128:
                                      hc * QC + ci * 128 + 128],
                                xT_sb[:, hc * S + j * 512:
                                      hc * S + j * 512 + 512],
                                start=(hc == 0), stop=(hc == HC - 1))
                            vt_mms[(ci, hc, j)] = mm
                    nc.vector.tensor_copy(vt_sb[:, ci * S:(ci + 1) * S], ps[:])
                for hc in range(2, HC):
                    tile.add_dep_helper(xt_dmas[hc].ins, vt_mms[(0, hc - 2, 3)].ins,
                                        reason="pace xT input load")
                for i, d in enumerate(qk_dmas):
                    src_mm = vt_mms[(0, min(i // 2, HC - 1), 1)]
                    tile.add_dep_helper(d.ins, src_mm.ins, reason="pace w input load")
                for ci in range(2):
                    d = nc.scalar.dma_start(
                        wo_sb[:, ci * HID:(ci + 1) * HID],
                        wo[ci * 128:(ci + 1) * 128, :])
                    tile.add_dep_helper(d.ins, vt_mms[(1, 3 + 2 * ci, 0)].ins,
                                        reason="pace wo load")

                # Q^T and K^T, written into the duplicated per-head layout
                for ci in range(2):
                    for w_sb, b_sb, dst in ((wq_sb, bq_sb, qt2),
                                            (wk_sb, bk_sb, kt2)):
                        ps = ps1.tile([128, S], F32, tag="ps1")
                        for hc in range(HC):
                            for j in range(TB):
                                nc.tensor.matmul(
                                    ps[:, j * 512:(j + 1) * 512],
                                    w_sb[:, hc * QC + ci * 128:
                                         hc * QC + ci * 128 + 128],
                                    xT_sb[:, hc * S + j * 512:
                                          hc * S + j * 512 + 512],
                                    start=(hc == 0), stop=(hc == HC - 1))
                        hA, hB = 2 * ci, 2 * ci + 1
                        nc.vector.tensor_scalar_add(
                            dst[0:64, hA * S:(hA + 1) * S], ps[0:64, :],
                            b_sb[0:64, ci:ci + 1])
                        nc.vector.tensor_scalar_add(
                            dst[64:128, hB * S:(hB + 1) * S], ps[64:128, :],
                            b_sb[64:128, ci:ci + 1])
                        nc.sync.dma_start(dst[64:128, hA * S:(hA + 1) * S],
                                          dst[0:64, hA * S:(hA + 1) * S])
                        nc.scalar.dma_start(dst[0:64, hB * S:(hB + 1) * S],
                                            dst[64:128, hB * S:(hB + 1) * S])

            # V^T -> V via PE transpose-mode ([128,128] pair tiles), then a
            # strided DVE copy into the ones-padded layout.
            with tc.tile_pool(name="trp", bufs=4, space="PSUM") as trp:
                for ci in range(2):
                    for t in range(TC):
                        tp = trp.tile([128, 128], BF16, tag="tr")
                        nc.tensor.transpose(
                            tp[:, :],
                            vt_sb[:, ci * S + t * 128:ci * S + t * 128 + 128],
                            idb_sb[:, :])
                        dst = v_sb[:, (t * HPC + 2 * ci) * 128:
                                   (t * HPC + 2 * ci + 2) * 128].rearrange(
                            "p (h e) -> p h e", h=2)[:, :, 0:HD]
                        srcv = tp[:, :].rearrange("p (h e) -> p h e", h=2)
                        nc.vector.tensor_copy(dst, srcv)

            # ---- phase 2: attention, q-block-major ------------------------
            # Units (j, h) are software-pipelined with LEAD=1: scores+exp of
            # unit k+1 are emitted before ctx of unit k so the PE's ctx block
            # never starves the Scalar engine's exp stream.  ctx accumulates
            # in two alternating PSUM banks (A: even chunks, B: odd chunks)
            # to avoid same-bank drain contention; the division fuses A+B.
            # Out-projection + output DMA for block j overlap block j+1.
            with (
                tc.tile_pool(name="probs", bufs=4) as probsp,
                tc.tile_pool(name="div", bufs=4) as divp,
                tc.tile_pool(name="ostg", bufs=3) as ostg,
                tc.tile_pool(name="scps", bufs=2, space="PSUM") as scps,
                tc.tile_pool(name="ctps", bufs=1, space="PSUM") as ctps,
                tc.tile_pool(name="ops", bufs=2, space="PSUM") as ops,
            ):
                units = [(j, h) for j in range(TB) for h in range(HPC)]
                probs_map = {}
                ctx_map = {}

                def emit_scores(j, h):
                    hS = h * S
                    probs_map[(j, h)] = probsp.tile(
                        [128, 4 * 1024], BF16, tag="probs",
                        name=f"probs_a_{j}_{h}")
                    for cp in range(TC // 2):
                        if cp == 4:
                            probs_map[(j, h, "b")] = probsp.tile(
                                [128, 4 * 1024], BF16, tag="probs",
                                name=f"probs_b_{j}_{h}")
                        c0, c1 = 2 * cp, 2 * cp + 1
                        sp = scps.tile([128, 1024], F32, tag="sc")
                        nc.tensor.matmul(
                            sp[:, 0:512],
                            kt2[0:64, hS + c0 * 128:hS + c0 * 128 + 128],
                            qt2[0:64, hS + j * 512:hS + j * 512 + 512],
                            start=True, stop=True, tile_position=(0, 0))
                        nc.tensor.matmul(
                            sp[:, 512:1024],
                            kt2[64:128, hS + c1 * 128:hS + c1 * 128 + 128],
                            qt2[64:128, hS + j * 512:hS + j * 512 + 512],
                            start=True, stop=True, tile_position=(64, 0))
                        pt = (probs_map[(j, h)] if cp < 4
                              else probs_map[(j, h, "b")])
                        o = (cp % 4) * 1024
                        nc.scalar.activation(pt[:, o:o + 1024], sp[:, :], EXP)

                def emit_ctx(j, h):
                    ctxA = ctps.tile([128, 512], F32, tag="ctxA")
                    ctxB = ctps.tile([128, 512], F32, tag="ctxB")
                    ctx_map[(j, h)] = (ctxA, ctxB)
                    pa = probs_map.pop((j, h))
                    pb = probs_map.pop((j, h, "b"))
                    for cp in range(TC // 2):
                        c0, c1 = 2 * cp, 2 * cp + 1
                        probs = pa if cp < 4 else pb
                        o = (cp % 4) * 1024
                        for ck, coff, cps in ((c0, 0, ctxA), (c1, 512, ctxB)):
                            strip = (ck * HPC + h) * 128
                            nc.tensor.matmul(
                                cps[0:HD + 1, :],
                                v_sb[:, strip:strip + HD + 1],
                                probs[:, o + coff:o + coff + 512],
                                start=(cp == 0), stop=(cp == TC // 2 - 1))

                def emit_division(j, h):
                    ci, lo = h // 2, (h % 2) * 64
                    ctxA, ctxB = ctx_map.pop((j, h))
                    cA = divp.tile([65, 512], F32, tag="cA")
                    nc.vector.tensor_copy(cA[0:65, :], ctxA[0:65, :])
                    craw = divp.tile([65, 512], F32, tag="craw")
                    nc.vector.tensor_tensor(out=craw[0:65, :],
                                            in0=cA[0:65, :],
                                            in1=ctxB[0:65, :],
                                            op=mybir.AluOpType.add)
                    denr = divp.tile([128, 4], F32, tag="denr")
                    nc.sync.dma_start(denr[:, :], craw[64:65, :])
                    recr = divp.tile([128, 4], F32, tag="recr")
                    nc.vector.reciprocal(recr[:, :], denr[:, :])
                    rrow = divp.tile([1, 512], F32, tag="rrow")
                    nc.sync.dma_start(rrow[:, :], recr[:, :])
                    Dt = divp.tile([64, 512], F32, tag="Dt")
                    nc.gpsimd.partition_broadcast(Dt[:, :], rrow[0:1, :])
                    o = ci * S + j * 512
                    if lo == 0:
                        nc.vector.tensor_tensor(
                            out=ctxf_sb[0:64, o:o + 512],
                            in0=craw[0:64, :], in1=Dt[:, :], op=MULT)
                    else:
                        ctxd = divp.tile([64, 512], BF16, tag="ctxd")
                        nc.vector.tensor_tensor(
                            out=ctxd[:, :], in0=craw[0:64, :],
                            in1=Dt[:, :], op=MULT)
                        nc.gpsimd.dma_start(ctxf_sb[64:128, o:o + 512],
                                            ctxd[:, :])

                def emit_outproj(j):
                    # all output DMAs on the sync queue: a gpsimd-queued
                    # 256KB store was observed to block partition_broadcast
                    # (same engine) and stall the next block's division
                    for tt in range(4):
                        t = 4 * j + tt
                        ot = ostg.tile([128, 1024], BF16, tag="ot")
                        for oc in range(2):
                            op = ops.tile([128, 512], F32, tag="op")
                            for ci in range(2):
                                nc.tensor.matmul(
                                    op[:, :],
                                    ctxf_sb[:, ci * S + t * 128:ci * S + t * 128 + 128],
                                    wo_sb[:, ci * HID + oc * 512:
                                          ci * HID + oc * 512 + 512],
                                    start=(ci == 0), stop=(ci == 1))
                            nc.vector.tensor_copy(
                                ot[:, oc * 512:(oc + 1) * 512], op[:, :])
                        nc.sync.dma_start(out[t * 128:(t + 1) * 128, :],
                                          ot[:, :])

                LEAD = 1
                for k in range(len(units) + LEAD):
                    if k < len(units):
                        emit_scores(*units[k])
                    if k >= LEAD:
                        j, h = units[k - LEAD]
                        emit_ctx(j, h)
                        emit_division(j, h)
                        if h == HPC - 1:
                            emit_outproj(j)

    nc.compile()
    return nc


_NC = None


def _get_nc():
    global _NC
    if _NC is None:
        _NC = build_nc()
    return _NC


def make_in_maps(x, Wq, bq, Wk, bk, Wv, bv, Wo, bo):
    in_maps = []
    for core in range(NCORES):
        b, g = core // 4, core % 4
        sl = slice(g * QC, (g + 1) * QC)
        in_maps.append({
            "xT": np.ascontiguousarray(x[b].T).astype(np.float16),
            "wq": (np.ascontiguousarray(Wq[:, sl]) * 0.125).astype(np.float16),
            "wk": np.ascontiguousarray(Wk[:, sl]).astype(np.float16),
            "wv": np.ascontiguousarray(Wv[:, sl]).astype(np.float16),
            "wo": np.ascontiguousarray(Wo[sl, :]).astype(ml_dtypes.bfloat16),
            "bq": (np.asarray(bq[sl]) * 0.125).astype(np.float32),
            "bk": np.asarray(bk[sl]).astype(np.float32),
            "ident": np.eye(128, dtype=np.float32),
        })
    return in_maps


def combine_outputs(core_outs, Wv_bias_term):
    full = np.empty((B, S, HID), np.float32)
    for b in range(B):
        acc = core_outs[4 * b].astype(np.float32).copy()
        for g in range(1, 4):
            acc += core_outs[4 * b + g]
        full[b] = acc + Wv_bias_term
    return full


def kernel(**inputs):
    x = np.asarray(inputs["x"], np.float32)
    Wq = np.asarray(inputs["Wq"], np.float32)
    bq = np.asarray(inputs["bq"], np.float32)
    Wk = np.asarray(inputs["Wk"], np.float32)
    bk = np.asarray(inputs["bk"], np.float32)
    Wv = np.asarray(inputs["Wv"], np.float32)
    bv = np.asarray(inputs["bv"], np.float32)
    Wo = np.asarray(inputs["Wo"], np.float32)
    bo = np.asarray(inputs["bo"], np.float32)

    nc = _get_nc()
    in_maps = make_in_maps(x, Wq, bq, Wk, bk, Wv, bv, Wo, bo)
    res = run_bass_kernel_spmd(nc, in_maps, core_ids=list(range(NCORES)))
    core_outs = [res.results[c]["out"] for c in range(NCORES)]
    bias_term = (bv @ Wo + bo).astype(np.float32)
    return combine_outputs(core_outs, bias_term)



# revision 8
# speedup vs baseline: 1.0232x; 1.0232x over previous
"""Multi-head attention (B=2, S=2048, H=1024, 16 heads) on 8 TRN2 NeuronCores.

Sharding (tensor-parallel heads x data-parallel batch, per the hint):
  core c -> batch b = c // 4, head group g = c % 4 (4 heads each).

v2 design (vs the v1 transpose-based kernel):
  - V is computed in natural orientation directly (stationary = x token
    chunks, moving = Wv) -- no PE transpose pass, no strided DVE unpack.
  - Attention runs head-major: per head, scores+exp for all four 512-token
    q-blocks stream through two PSUM score tiles while ctx accumulates into
    FOUR per-block PSUM banks, k-chunk-major, so each V-strip LDWEIGHTS is
    reused by 4 matmuls and consecutive ctx matmuls never share a bank
    (kills the v1 ctxA/ctxB merge pass entirely).
  - exp is split across two engines: ~5/8 of chunk-pairs on ScalarE
    (table exp, scale=1/ALPHA bias=-CSHIFT), ~3/8 on VectorE via a
    Schraudolph bf16 exp: probs_bits = int16(alpha*s + beta), where
    alpha = 128*log2(e) is folded into Wq on the host, so the DVE op is a
    single tensor_scalar(add beta, max 0) with int16 output bitcast onto
    the bf16 probs tile.  The softmax denominator (the M=65 ones-column in
    the ctx stationary) is built from the same approximate probs, so the
    systematic Schraudolph error largely cancels in the division.
  - Out-projection runs as a tail after the last division, with the
    PSUM->SBUF copies split between ScalarE and VectorE.

Numerics: fp16 projections (f32 PSUM accum), fp16 scores operands, bf16
probs/V/ctx.  Full-model emulation of this scheme measures rel_err ~9.5e-3
(gate is 2e-2).  bq/bk applied on device (scaled by ALPHA for bq);
bv/bo folded into a host-side additive constant (exact).
"""

import ml_dtypes
import numpy as np

import concourse.bacc as bacc
import concourse.mybir as mybir
import concourse.tile as tile
from concourse.bass_utils import run_bass_kernel_spmd

NCORES = 8
B, S, HID = 2, 2048, 1024
NH, HD = 16, 64
HPC = 4            # heads per core
QC = HPC * HD      # 256 local projection cols per core
HC = HID // 128    # 8 hidden chunks
TC = S // 128      # 16 token chunks
TB = S // 512      # 4 token blocks

ALPHA = 128.0 * np.log2(np.e)                    # 184.6637 folded into Wq
CSHIFT = 2.0                                     # probs scaled by e^-CSHIFT
BETA = 16256.0 - CSHIFT * ALPHA + 0.5 - 5.57     # bias + trunc comp + centering
VSTRIDE = HPC * (HD + 1)                         # 260: v_sb cols per token chunk

F32 = mybir.dt.float32
BF16 = mybir.dt.bfloat16
FP16 = mybir.dt.float16
I16 = mybir.dt.int16
EXP = mybir.ActivationFunctionType.Exp
MULT = mybir.AluOpType.mult
ADD = mybir.AluOpType.add
MAX = mybir.AluOpType.max

# chunk-pair (cp, j) -> True if exp goes to the DVE Schraudolph path
def _on_dve(cp, j):
    return (cp * HPC + j) % 8 in (2, 5, 7)


def build_nc():
    nc = bacc.Bacc("TRN2", target_bir_lowering=False, debug=False,
                   num_devices=NCORES)
    xT = nc.declare_dram_parameter("xT", [HID, S], FP16, isOutput=False)
    wq = nc.declare_dram_parameter("wq", [HID, QC], FP16, isOutput=False)
    wk = nc.declare_dram_parameter("wk", [HID, QC], FP16, isOutput=False)
    wv = nc.declare_dram_parameter("wv", [HID, QC], FP16, isOutput=False)
    wo = nc.declare_dram_parameter("wo", [QC, HID], BF16, isOutput=False)
    bq = nc.declare_dram_parameter("bq", [QC], F32, isOutput=False)
    bk = nc.declare_dram_parameter("bk", [QC], F32, isOutput=False)
    out = nc.declare_dram_parameter("out", [S, HID], BF16, isOutput=True)

    with tile.TileContext(nc) as tc:
        with (
            tc.tile_pool(name="const", bufs=1) as constp,
            tc.tile_pool(name="qkv", bufs=1) as qkvp,
        ):
            wo_sb = constp.tile([128, 2 * HID], BF16)
            bq_sb = constp.tile([128, 2], F32)
            bk_sb = constp.tile([128, 2], F32)
            warm = constp.tile([1, 8], F32)
            nbias = constp.tile([128, 1], F32)
            nc.vector.memset(nbias[:], -CSHIFT)
            # Q^T/K^T per head, duplicated across both partition halves so the
            # scores matmuls run two tok_k chunks concurrently as row-tiles.
            qt2 = qkvp.tile([128, HPC * S], FP16)
            kt2 = qkvp.tile([128, HPC * S], FP16)
            # Natural V (bf16): per token chunk t, 4 strips [128, 65]
            # (64 v cols + a ones col at 64 -> softmax denominator lands in
            # ctx row 64 of the same accumulation).
            v_sb = qkvp.tile([128, TC * VSTRIDE], BF16)
            ctxf_sb = qkvp.tile([128, 2 * S], BF16)

            # preload the exp table set while DMAs run
            nc.vector.memset(warm[:], 0.0)
            nc.scalar.activation(warm[:], warm[:], EXP)
            # ones columns: set everything to 1.0, V copies overwrite cols 0:64
            nc.vector.memset(v_sb[:], 1.0)

            # ---- phase 1: projections -------------------------------------
            with tc.tile_pool(name="xw", bufs=1) as xwp:
                xT_sb = xwp.tile([128, HC * S], FP16)
                wq_sb = xwp.tile([128, HC * QC], FP16)
                wk_sb = xwp.tile([128, HC * QC], FP16)
                wv_sb = xwp.tile([128, HC * QC], FP16)

                # wv first, then xT in j-quarters so V chunk 0 starts asap
                for hc in range(HC):
                    r = slice(hc * 128, (hc + 1) * 128)
                    eng = nc.scalar if hc % 2 == 0 else nc.sync
                    eng.dma_start(wv_sb[:, hc * QC:(hc + 1) * QC], wv[r, :])
                xt_dmas = {}
                for jq in range(TB):
                    for hc in range(HC):
                        r = slice(hc * 128, (hc + 1) * 128)
                        eng = nc.sync if hc % 2 == 0 else nc.scalar
                        xt_dmas[(jq, hc)] = eng.dma_start(
                            xT_sb[:, hc * S + jq * 512:hc * S + (jq + 1) * 512],
                            xT[r, jq * 512:(jq + 1) * 512])
                for ci in range(2):
                    nc.sync.dma_start(bq_sb[:, ci:ci + 1],
                                      bq[ci * 128:(ci + 1) * 128])
                    nc.sync.dma_start(bk_sb[:, ci:ci + 1],
                                      bk[ci * 128:(ci + 1) * 128])
                qk_dmas = []
                for hc in range(HC):
                    r = slice(hc * 128, (hc + 1) * 128)
                    qk_dmas.append(nc.sync.dma_start(
                        wq_sb[:, hc * QC:(hc + 1) * QC], wq[r, :]))
                    qk_dmas.append(nc.scalar.dma_start(
                        wk_sb[:, hc * QC:(hc + 1) * QC], wk[r, :]))

                # V natural: stationary = x token chunk, moving = Wv
                v_mms = {}
                with tc.tile_pool(name="vps", bufs=4, space="PSUM") as vps:
                    for t in range(TC):
                        vp = vps.tile([128, QC], F32, tag="vps")
                        for hc in range(HC):
                            mm = nc.tensor.matmul(
                                vp[:, :],
                                xT_sb[:, hc * S + t * 128:hc * S + t * 128 + 128],
                                wv_sb[:, hc * QC:(hc + 1) * QC],
                                start=(hc == 0), stop=(hc == HC - 1))
                            v_mms[(t, hc)] = mm
                        dst = v_sb[:, t * VSTRIDE:(t + 1) * VSTRIDE].rearrange(
                            "p (h e) -> p h e", h=HPC)[:, :, 0:HD]
                        src = vp[:, :].rearrange("p (h e) -> p h e", h=HPC)
                        nc.vector.tensor_copy(dst, src)

                # pace the w / wo input loads behind early V matmuls
                for i, d in enumerate(qk_dmas):
                    src_mm = v_mms[(min(2 + i // 2, TC - 1), 0)]
                    tile.add_dep_helper(d.ins, src_mm.ins,
                                        reason="pace w input load")
                for ci in range(2):
                    d = nc.gpsimd.dma_start(
                        wo_sb[:, ci * HID:(ci + 1) * HID],
                        wo[ci * 128:(ci + 1) * 128, :])
                    tile.add_dep_helper(d.ins, v_mms[(10 + 2 * ci, 0)].ins,
                                        reason="pace wo load")

                # Q^T and K^T (psum [128, S] per (proj, ci)), written into the
                # duplicated per-head layout.  ALPHA is folded into wq/bq.
                with tc.tile_pool(name="ps1", bufs=2, space="PSUM") as ps1:
                    for ci in range(2):
                        for w_sb, b_sb, dst in ((wq_sb, bq_sb, qt2),
                                                (wk_sb, bk_sb, kt2)):
                            ps = ps1.tile([128, S], F32, tag="ps1")
                            for hc in range(HC):
                                for j in range(TB):
                                    nc.tensor.matmul(
                                        ps[:, j * 512:(j + 1) * 512],
                                        w_sb[:, hc * QC + ci * 128:
                                             hc * QC + ci * 128 + 128],
                                        xT_sb[:, hc * S + j * 512:
                                              hc * S + j * 512 + 512],
                                        start=(hc == 0), stop=(hc == HC - 1))
                            hA, hB = 2 * ci, 2 * ci + 1
                            nc.vector.tensor_scalar_add(
                                dst[0:64, hA * S:(hA + 1) * S], ps[0:64, :],
                                b_sb[0:64, ci:ci + 1])
                            nc.vector.tensor_scalar_add(
                                dst[64:128, hB * S:(hB + 1) * S], ps[64:128, :],
                                b_sb[64:128, ci:ci + 1])
                            nc.sync.dma_start(dst[64:128, hA * S:(hA + 1) * S],
                                              dst[0:64, hA * S:(hA + 1) * S])
                            nc.scalar.dma_start(dst[0:64, hB * S:(hB + 1) * S],
                                                dst[64:128, hB * S:(hB + 1) * S])

            # ---- phase 2: attention, head-major ---------------------------
            with (
                tc.tile_pool(name="probs", bufs=1) as probsp,
                tc.tile_pool(name="div", bufs=4) as divp,
                tc.tile_pool(name="scps", bufs=2, space="PSUM") as scps,
                tc.tile_pool(name="ctps", bufs=1, space="PSUM") as ctps,
            ):
                probs = [probsp.tile([128, 8 * 1024], BF16, tag=f"probs{j}",
                                     name=f"probs{j}")
                         for j in range(TB)]

                def emit_scores_slot(h, cp):
                    """scores + exp for chunk pair (2cp, 2cp+1), all 4 q-blocks."""
                    hS = h * S
                    c0, c1 = 2 * cp, 2 * cp + 1
                    for j in range(TB):
                        sp = scps.tile([128, 1024], F32, tag="sc")
                        nc.tensor.matmul(
                            sp[:, 0:512],
                            kt2[0:64, hS + c0 * 128:hS + c0 * 128 + 128],
                            qt2[0:64, hS + j * 512:hS + j * 512 + 512],
                            start=True, stop=True, tile_position=(0, 0))
                        nc.tensor.matmul(
                            sp[:, 512:1024],
                            kt2[64:128, hS + c1 * 128:hS + c1 * 128 + 128],
                            qt2[64:128, hS + j * 512:hS + j * 512 + 512],
                            start=True, stop=True, tile_position=(64, 0))
                        dst = probs[j][:, cp * 1024:(cp + 1) * 1024]
                        if _on_dve(cp, j):
                            nc.vector.tensor_scalar(
                                out=dst.bitcast(I16), in0=sp[:, :],
                                scalar1=BETA, scalar2=0.0, op0=ADD, op1=MAX)
                        else:
                            nc.scalar.activation(dst, sp[:, :], EXP,
                                                 bias=nbias[:, 0:1],
                                                 scale=1.0 / ALPHA)

                ctx_map = {}

                def emit_ctx_pair(h, cp):
                    """ctx chunk-pair: LDW each strip once, 4 q-block matmuls."""
                    for c in (2 * cp, 2 * cp + 1):
                        strip = v_sb[:, c * VSTRIDE + h * (HD + 1):
                                     c * VSTRIDE + h * (HD + 1) + HD + 1]
                        for j in range(TB):
                            if cp == 0 and c == 0:
                                ctx_map[(h, j)] = ctps.tile(
                                    [128, 512], F32, tag=f"ctx{j}",
                                    name=f"ctx_{h}_{j}")
                            nc.tensor.matmul(
                                ctx_map[(h, j)][0:HD + 1, :],
                                strip,
                                probs[j][:, c * 512:(c + 1) * 512],
                                start=(c == 0), stop=(c == TC - 1))

                def emit_division(h, j):
                    ci, lo = h // 2, (h % 2) * 64
                    craw = ctx_map.pop((h, j))
                    dn = divp.tile([65, 512], F32, tag="dn")
                    nc.vector.tensor_copy(dn[64:65, :], craw[64:65, :])
                    denr = divp.tile([128, 4], F32, tag="denr")
                    nc.sync.dma_start(denr[:, :], dn[64:65, :])
                    recr = divp.tile([128, 4], F32, tag="recr")
                    nc.vector.reciprocal(recr[:, :], denr[:, :])
                    rrow = divp.tile([1, 512], F32, tag="rrow")
                    nc.sync.dma_start(rrow[:, :], recr[:, :])
                    Dt = divp.tile([64, 512], F32, tag="Dt")
                    nc.gpsimd.partition_broadcast(Dt[:, :], rrow[0:1, :])
                    o = ci * S + j * 512
                    if lo == 0:
                        nc.vector.tensor_tensor(
                            out=ctxf_sb[0:64, o:o + 512],
                            in0=craw[0:64, :], in1=Dt[:, :], op=MULT)
                    else:
                        ctxd = divp.tile([64, 512], BF16, tag="ctxd")
                        nc.vector.tensor_tensor(
                            out=ctxd[:, :], in0=craw[0:64, :],
                            in1=Dt[:, :], op=MULT)
                        nc.gpsimd.dma_start(ctxf_sb[64:128, o:o + 512],
                                            ctxd[:, :])

                for h in range(HPC):
                    emit_scores_slot(h, 0)
                    for cp in range(1, TC // 2):
                        emit_scores_slot(h, cp)
                        emit_ctx_pair(h, cp - 1)
                    emit_ctx_pair(h, TC // 2 - 1)
                    for j in range(TB):
                        emit_division(h, j)

            # ---- phase 3: out-projection tail -----------------------------
            with (
                tc.tile_pool(name="ostg", bufs=3) as ostg,
                tc.tile_pool(name="ops", bufs=4, space="PSUM") as ops,
            ):
                for t in range(TC):
                    ot = ostg.tile([128, 1024], BF16, tag="ot")
                    for oc in range(2):
                        op = ops.tile([128, 512], F32, tag="op")
                        for ci in range(2):
                            nc.tensor.matmul(
                                op[:, :],
                                ctxf_sb[:, ci * S + t * 128:ci * S + t * 128 + 128],
                                wo_sb[:, ci * HID + oc * 512:
                                      ci * HID + oc * 512 + 512],
                                start=(ci == 0), stop=(ci == 1))
                        eng = nc.vector if oc == 0 else nc.scalar
                        if oc == 0:
                            nc.vector.tensor_copy(
                                ot[:, oc * 512:(oc + 1) * 512], op[:, :])
                        else:
                            nc.scalar.copy(
                                ot[:, oc * 512:(oc + 1) * 512], op[:, :])
                    nc.sync.dma_start(out[t * 128:(t + 1) * 128, :], ot[:, :])

    nc.compile()
    return nc


_NC = None


def _get_nc():
    global _NC
    if _NC is None:
        _NC = build_nc()
    return _NC


def make_in_maps(x, Wq, bq, Wk, bk, Wv, bv, Wo, bo):
    qscale = 0.125 * ALPHA
    in_maps = []
    for core in range(NCORES):
        b, g = core // 4, core % 4
        sl = slice(g * QC, (g + 1) * QC)
        in_maps.append({
            "xT": np.ascontiguousarray(x[b].T).astype(np.float16),
            "wq": (np.ascontiguousarray(Wq[:, sl]) * qscale).astype(np.float16),
            "wk": np.ascontiguousarray(Wk[:, sl]).astype(np.float16),
            "wv": np.ascontiguousarray(Wv[:, sl]).astype(np.float16),
            "wo": np.ascontiguousarray(Wo[sl, :]).astype(ml_dtypes.bfloat16),
            "bq": (np.asarray(bq[sl]) * qscale).astype(np.float32),
            "bk": np.asarray(bk[sl]).astype(np.float32),
        })
    return in_maps


def combine_outputs(core_outs, Wv_bias_term):
    full = np.empty((B, S, HID), np.float32)
    for b in range(B):
        acc = core_outs[4 * b].astype(np.float32).copy()
        for g in range(1, 4):
            acc += core_outs[4 * b + g]
        full[b] = acc + Wv_bias_term
    return full


def kernel(**inputs):
    x = np.asarray(inputs["x"], np.float32)
    Wq = np.asarray(inputs["Wq"], np.float32)
    bq = np.asarray(inputs["bq"], np.float32)
    Wk = np.asarray(inputs["Wk"], np.float32)
    bk = np.asarray(inputs["bk"], np.float32)
    Wv = np.asarray(inputs["Wv"], np.float32)
    bv = np.asarray(inputs["bv"], np.float32)
    Wo = np.asarray(inputs["Wo"], np.float32)
    bo = np.asarray(inputs["bo"], np.float32)

    nc = _get_nc()
    in_maps = make_in_maps(x, Wq, bq, Wk, bk, Wv, bv, Wo, bo)
    res = run_bass_kernel_spmd(nc, in_maps, core_ids=list(range(NCORES)))
    core_outs = [res.results[c]["out"] for c in range(NCORES)]
    bias_term = (bv @ Wo + bo).astype(np.float32)
    return combine_outputs(core_outs, bias_term)
